# revision 6
# baseline (speedup 1.0000x reference)
"""BiGRU encoder (2 layers, bidirectional, S=256 B=64 H=512) on 8 TRN2 cores.

Sharding: core = (direction d, batch quarter q); each core gathers its
embedding slice, precomputes input-side gate terms U = x @ Wx.T + b as
large matmuls, then runs the 256-step recurrences for both layers with
weight-stationary bf16 matmuls in feature-major layout [128p, 4k, cols].
h is carried in fp32; returns (out [256,64,1024], hidd [4,64,512]) fp32.
"""

import sys

sys.path.insert(0, "/opt/trn_rl_repo")

import numpy as np
import ml_dtypes

H = 512
KT = 4
NW = 32000
GATE_M = 12
S = 256
B = 16  # per-core batch
U_COLS = 512

LAST_EXEC_NS = None
_CACHE = {}


def _build(S=S):
    from concourse import mybir, bacc
    from concourse.tile import TileContext
    from concourse.bass import ts

    nc = bacc.Bacc(None)
    N = S * B
    u_cols = min(U_COLS, N)
    n_uchunks = N // u_cols
    spc = u_cols // B  # steps per chunk

    emb_p = nc.declare_dram_parameter("emb16", [NW, H], mybir.dt.bfloat16, isOutput=False)
    idx_p = nc.declare_dram_parameter("idx", [128, N // 16], mybir.dt.int16, isOutput=False)
    wparams = {}
    for nm in ("wu1", "wh1", "wu2", "wh2"):
        wparams[nm] = nc.declare_dram_parameter(nm, [128, KT, 3 * H], mybir.dt.bfloat16, isOutput=False)
    bparams = {}
    for nm in ("ub1", "ub2"):
        bparams[nm] = nc.declare_dram_parameter(nm, [128, GATE_M], mybir.dt.float32, isOutput=False)
    out_p = nc.declare_dram_parameter("out_o", [128, KT, N], mybir.dt.float32, isOutput=True)
    hid_p = nc.declare_dram_parameter("hid_o", [2, 128, KT, B], mybir.dt.float32, isOutput=True)

    u_dram = nc.dram_tensor("u_dram", [128, GATE_M, N], mybir.dt.float32)

    f32 = mybir.dt.float32
    bf16 = mybir.dt.bfloat16
    AF = mybir.ActivationFunctionType

    with TileContext(nc) as tc:
        with (
            tc.tile_pool(name="const", bufs=1) as constp,
            tc.tile_pool(name="seq", bufs=2) as seqp,
            tc.tile_pool(name="uc", bufs=2) as ucp,
            tc.tile_pool(name="o2c", bufs=2) as o2p,
            tc.tile_pool(name="st", bufs=2) as stp,
            tc.tile_pool(name="hh", bufs=1) as hhp,
            tc.tile_pool(name="pu", bufs=2, space="PSUM") as pup,
            tc.tile_pool(name="pg", bufs=2, space="PSUM") as pgp,
        ):
            idx_t = constp.tile([128, N // 16], mybir.dt.int16, tag="idx")
            nc.sync.dma_start(out=idx_t[:], in_=idx_p[:])
            w_t = {}
            for nm in ("wu1", "wh1", "wu2", "wh2"):
                w_t[nm] = constp.tile([128, KT, 3 * H], bf16, tag=nm, name=nm)
                nc.sync.dma_start(out=w_t[nm][:], in_=wparams[nm][:])
            ub_t = {}
            for nm in ("ub1", "ub2"):
                ub_t[nm] = constp.tile([128, GATE_M], f32, tag=nm, name=nm)
                nc.sync.dma_start(out=ub_t[nm][:], in_=bparams[nm][:])

            x_t = seqp.tile([128, n_uchunks, KT, u_cols], bf16, tag="seq")
            for g in range(n_uchunks):
                nc.gpsimd.dma_gather(
                    out_ap=x_t[:, g], in_ap=emb_p[:],
                    idxs_ap=idx_t[:, ts(g, u_cols // 16)],
                    num_idxs=u_cols, num_idxs_reg=u_cols, elem_size=H, transpose=True,
                )
            out1_t = seqp.tile([128, n_uchunks, KT, u_cols], bf16, tag="seq")

            def emit_U(src_t, wu, ub):
                for c in range(n_uchunks):
                    ust = ucp.tile([128, GATE_M, u_cols], f32, tag="uc")
                    for m in range(GATE_M):
                        psu = pup.tile([128, u_cols], f32, tag="pu")
                        for k in range(KT):
                            nc.tensor.matmul(
                                out=psu[:], lhsT=wu[:, k, ts(m, 128)],
                                rhs=src_t[:, c, k, :],
                                start=(k == 0), stop=(k == KT - 1),
                            )
                        nc.scalar.activation(
                            out=ust[:, m, :], in_=psu[:], func=AF.Identity,
                            bias=ub[:, m : m + 1], scale=1.0,
                        )
                    nc.sync.dma_start(out=u_dram[:, :, ts(c, u_cols)], in_=ust[:])

            def run_layer(layer, wh, store_seq):
                h = hhp.tile([128, KT, B], f32, tag=f"h{layer}", name=f"h{layer}")
                hb = hhp.tile([128, KT, B], bf16, tag=f"hb{layer}", name=f"hb{layer}")
                nc.vector.memset(h[:], 0.0)
                nc.vector.memset(hb[:], 0.0)
                for c in range(n_uchunks):
                    ucT = ucp.tile([128, GATE_M, u_cols], f32, tag="uc")
                    nc.sync.dma_start(out=ucT[:], in_=u_dram[:, :, ts(c, u_cols)])
                    o2 = None
                    if layer == 1:
                        o2 = o2p.tile([128, KT, u_cols], f32, tag="o2c")
                    for tt in range(spc):
                        t = c * spc + tt
                        ps_r = pgp.tile([128, KT, B], f32, tag="ps_r")
                        ps_z = pgp.tile([128, KT, B], f32, tag="ps_z")
                        ps_s = pgp.tile([128, KT, B], f32, tag="ps_s")
                        for m in range(KT):
                            for k in range(KT):
                                nc.tensor.matmul(
                                    out=ps_r[:, m, :], lhsT=wh[:, k, ts(m, 128)],
                                    rhs=hb[:, k, :], start=(k == 0), stop=(k == KT - 1),
                                )
                        for m in range(KT):
                            for k in range(KT):
                                nc.tensor.matmul(
                                    out=ps_z[:, m, :],
                                    lhsT=wh[:, k, 512 + m * 128 : 512 + (m + 1) * 128],
                                    rhs=hb[:, k, :], start=(k == 0), stop=(k == KT - 1),
                                )
                        pre_r = stp.tile([128, KT, B], f32, tag="pre_r")
                        nc.vector.tensor_add(pre_r[:], ps_r[:], ucT[:, 0:KT, ts(tt, B)])
                        r_bf = stp.tile([128, KT, B], bf16, tag="r_bf")
                        nc.scalar.activation(out=r_bf[:], in_=pre_r[:], func=AF.Sigmoid)
                        rh = stp.tile([128, KT, B], bf16, tag="rh")
                        nc.vector.tensor_mul(rh[:], r_bf[:], hb[:])
                        for m in range(KT):
                            for k in range(KT):
                                nc.tensor.matmul(
                                    out=ps_s[:, m, :],
                                    lhsT=wh[:, k, 1024 + m * 128 : 1024 + (m + 1) * 128],
                                    rhs=rh[:, k, :], start=(k == 0), stop=(k == KT - 1),
                                )
                        pre_z = stp.tile([128, KT, B], f32, tag="pre_z")
                        nc.vector.tensor_add(pre_z[:], ps_z[:], ucT[:, KT : 2 * KT, ts(tt, B)])
                        zc = stp.tile([128, KT, B], f32, tag="zc")
                        nc.scalar.activation(out=zc[:], in_=pre_z[:], func=AF.Sigmoid, scale=-1.0)
                        pre_s = stp.tile([128, KT, B], f32, tag="pre_s")
                        nc.vector.tensor_add(pre_s[:], ps_s[:], ucT[:, 2 * KT : 3 * KT, ts(tt, B)])
                        s_t = stp.tile([128, KT, B], f32, tag="s_t")
                        nc.scalar.activation(out=s_t[:], in_=pre_s[:], func=AF.Tanh)
                        d = stp.tile([128, KT, B], f32, tag="d")
                        nc.vector.tensor_sub(d[:], s_t[:], h[:])
                        e = stp.tile([128, KT, B], f32, tag="e")
                        nc.vector.tensor_mul(e[:], zc[:], d[:])
                        nc.vector.tensor_add(h[:], h[:], e[:])
                        nc.vector.tensor_copy(out=hb[:], in_=h[:])
                        if layer == 0:
                            nc.vector.tensor_copy(out=store_seq[:, c, :, ts(tt, B)], in_=hb[:])
                        else:
                            nc.vector.tensor_copy(out=o2[:, :, ts(tt, B)], in_=h[:])
                    if layer == 1:
                        nc.sync.dma_start(out=out_p[:, :, ts(c, u_cols)], in_=o2[:])
                nc.sync.dma_start(out=hid_p[layer], in_=h[:])

            emit_U(x_t, w_t["wu1"], ub_t["ub1"])
            run_layer(0, w_t["wh1"], out1_t)
            emit_U(out1_t, w_t["wu2"], ub_t["ub2"])
            run_layer(1, w_t["wh2"], None)

    nc.finalize()
    return nc


def _prep_core_inputs(tokens, emb16, Wr, Wz, Ws, br, bz, bsv, d, q, S=S):
    tok = np.asarray(tokens)[:, q * B : (q + 1) * B].astype(np.int64)
    if d == 1:
        tok = tok[::-1]
    flat = tok.reshape(-1).astype(np.int16)
    n = flat.shape[0]
    wrapped = flat.reshape(n // 16, 16).T
    idx = np.ascontiguousarray(np.tile(wrapped, (8, 1)))

    def wmat(l, part):
        Wall = np.concatenate(
            [Wr[d, l][:, part * H : (part + 1) * H],
             Wz[d, l][:, part * H : (part + 1) * H],
             Ws[d, l][:, part * H : (part + 1) * H]], axis=0
        )
        WT = Wall.T.reshape(KT, 128, 3 * H)
        return np.ascontiguousarray(WT.transpose(1, 0, 2)).astype(ml_dtypes.bfloat16)

    def bias(l):
        b = np.stack([br[d, l], bz[d, l], bsv[d, l]])
        return np.ascontiguousarray(b.reshape(GATE_M, 128).T).astype(np.float32)

    return {
        "emb16": emb16, "idx": idx,
        "wu1": wmat(0, 0), "wh1": wmat(0, 1),
        "wu2": wmat(1, 0), "wh2": wmat(1, 1),
        "ub1": bias(0), "ub2": bias(1),
    }


def kernel(tokens, emb, Wr, Wz, Ws, br, bz, bsv):
    global LAST_EXEC_NS
    from concourse.bass_utils import run_bass_kernel_spmd

    if "nc" not in _CACHE:
        _CACHE["nc"] = _build()
    nc = _CACHE["nc"]

    tokens = np.asarray(tokens)
    emb = np.asarray(emb, dtype=np.float32)
    Wr = np.asarray(Wr, dtype=np.float32)
    Wz = np.asarray(Wz, dtype=np.float32)
    Ws = np.asarray(Ws, dtype=np.float32)
    br = np.asarray(br, dtype=np.float32)
    bz = np.asarray(bz, dtype=np.float32)
    bsv = np.asarray(bsv, dtype=np.float32)

    emb16 = np.ascontiguousarray(emb.astype(ml_dtypes.bfloat16))
    in_maps = [
        _prep_core_inputs(tokens, emb16, Wr, Wz, Ws, br, bz, bsv, ci // 4, ci % 4)
        for ci in range(8)
    ]
    res = run_bass_kernel_spmd(nc, in_maps, list(range(8)))
    if res.exec_time_ns is not None:
        LAST_EXEC_NS = res.exec_time_ns

    out = np.empty((S, 64, 1024), np.float32)
    hidd = np.empty((4, 64, H), np.float32)
    for ci, r in enumerate(res.results):
        d, q = ci // 4, ci % 4
        o = r["out_o"].reshape(128, KT, S, B)
        out[:, q * B : (q + 1) * B, d * H : (d + 1) * H] = (
            o.transpose(2, 3, 1, 0).reshape(S, B, H)
        )
        hid = r["hid_o"]
        for l in range(2):
            hidd[d * 2 + l, q * B : (q + 1) * B, :] = (
                hid[l].transpose(2, 1, 0).reshape(B, H)
            )
    return out, hidd
